# revision 1
# baseline (speedup 1.0000x reference)
"""Trainium2 Bass kernel for nn_MultiHeadAttention_66872640799208.

Math (per batch element b, S=2048, D=1024):
    qp = q @ Wq.T + bq ; kp = k @ Wk.T + bk ; vp = v @ Wv.T + bv
    scores = qp @ kp.T / D
    probs  = softmax(scores, axis=q)          # over the QUERY axis
    attn   = probs @ vp
    attn_w = softmax(attn, axis=q)            # over the sequence axis
    out    = (attn + q, attn_w)

Sharding: data-parallel over batch B=8 -> one batch element per NeuronCore,
no collectives. Host pre-transposes inputs to bf16 so that every matmul
contracts over the partition axis with no on-chip transposes:
  - qT/kT/vT [D, S] feed the projections (contraction over d),
  - qpT/kpT are produced in [e, s] layout so scoresT = kpT.T @ qpT has the
    softmax axis (q) on the free dimension,
  - vp is produced in natural [s, e] layout as lhsT/rhs of the attn matmul,
  - the probs 1/Z normalization (Z indexed by k) is folded into vp's rows
    (k is vp's partition axis) instead of scaling the much larger probs.
The second softmax (over q, the partition axis of attn) uses a ones-vector
PE matmul for the column sums and a K=1 PE matmul to broadcast 1/Z.

SBUF (192KB/partition budget) is managed with tag-slot reuse in one pool:
  tag A: qp (P1-P2) -> expb=exp(attn) (P3-P4)        32KB
  tag B: kp (P1-P2)                                  32KB
  tag W: weights (P1) -> probs (P2-P3)               64KB (max)
  tag V: vp (P1-P3)                                  32KB
"""

import sys

if "/opt/trn_rl_repo" not in sys.path:
    sys.path.insert(0, "/opt/trn_rl_repo")

import numpy as np
import ml_dtypes

B = 8
S = 2048
D = 1024
P = 128


def build_nc(s=S, d=D):
    """Build the single-core Bass program (SPMD: identical on all cores)."""
    import concourse.bass as bass
    import concourse.tile as tile
    from concourse import bacc, mybir

    bf16 = mybir.dt.bfloat16
    f32 = mybir.dt.float32

    DT = d // P          # contraction tiles for projections
    ET = d // P          # e (feature) tiles
    ST = s // P          # sequence tiles
    NFQ = min(512, s)    # matmul moving free-dim over q
    QC = s // NFQ        # q chunks
    NFD = min(512, d)    # matmul moving free-dim over d/e
    EC = d // NFD        # e chunks
    SCW = min(512, s)    # vT stream chunk width (in s)
    SC = s // SCW

    nc = bacc.Bacc("TRN2")

    qT = nc.dram_tensor("qT", [d, s], bf16, kind="ExternalInput")
    kT = nc.dram_tensor("kT", [d, s], bf16, kind="ExternalInput")
    vT = nc.dram_tensor("vT", [d, s], bf16, kind="ExternalInput")
    wqT = nc.dram_tensor("wqT", [d, d], bf16, kind="ExternalInput")  # [d, e]
    wkT = nc.dram_tensor("wkT", [d, d], bf16, kind="ExternalInput")
    wvT = nc.dram_tensor("wvT", [d, d], bf16, kind="ExternalInput")
    bq = nc.dram_tensor("bq", [d], f32, kind="ExternalInput")
    bk = nc.dram_tensor("bk", [d], f32, kind="ExternalInput")
    bv = nc.dram_tensor("bv", [d], f32, kind="ExternalInput")
    qres = nc.dram_tensor("qres", [s, d], f32, kind="ExternalInput")
    attn_o = nc.dram_tensor("attn", [s, d], f32, kind="ExternalOutput")
    attnw_o = nc.dram_tensor("attn_w", [s, d], f32, kind="ExternalOutput")

    qT_r = qT[:].rearrange("(dt p) s -> p dt s", p=P)
    kT_r = kT[:].rearrange("(dt p) s -> p dt s", p=P)
    vT_r = vT[:].rearrange("(dt p) s -> p dt s", p=P)
    w_rs = [
        w[:].rearrange("(dt p) e -> p dt e", p=P) for w in (wqT, wkT, wvT)
    ]
    bq_r = bq[:].rearrange("(t p) -> p t", p=P)
    bk_r = bk[:].rearrange("(t p) -> p t", p=P)
    qres_r = qres[:].rearrange("(st p) d -> p st d", p=P)
    attn_r = attn_o[:].rearrange("(st p) d -> p st d", p=P)
    attnw_r = attnw_o[:].rearrange("(st p) d -> p st d", p=P)

    with tile.TileContext(nc) as tc:
        with (
            tc.tile_pool(name="consts", bufs=1) as consts,
            tc.tile_pool(name="big", bufs=1) as big,
            tc.tile_pool(name="io", bufs=2) as io,
            tc.tile_pool(name="small", bufs=4) as small,
            tc.tile_pool(name="psum", bufs=4, space="PSUM") as psum,
            tc.tile_pool(name="psum1", bufs=1, space="PSUM") as psum1,
        ):
            # ---- constants (DMA order matters: wq + first q-chunk first so
            # the PE can start projecting ~8us in; the rest stream behind) ----
            wall = big.tile([P, 3, DT, d], bf16, tag="W")   # wq|wk|wv
            nc.sync.dma_start(out=wall[:, 0], in_=w_rs[0])
            bq_t = consts.tile([P, ET], f32)
            nc.sync.dma_start(out=bq_t[:], in_=bq_r)
            bk_t = consts.tile([P, ET], f32)
            bv_bc = consts.tile([P, d], f32)
            ones_col = consts.tile([P, 1], bf16)   # lhsT for column sums (K=P, M=1)
            nc.vector.memset(ones_col[:], 1.0)
            rz_all = consts.tile([P, ST], f32)     # per-k-row 1/Z of softmax #1
            rz2 = consts.tile([1, d], f32)         # 1/colsum of softmax #2
            ones_row = consts.tile([1, P], f32)    # lhsT for 1/Z broadcast (K=1)
            nc.vector.memset(ones_row[:], 1.0)

            qp = big.tile([P, ET, s], bf16, tag="A")        # qpT: [e, q]
            kp = big.tile([P, ET, s], bf16, tag="B")        # kpT: [e, k]
            vp = big.tile([P, ST, d], bf16, tag="V")        # natural [s, e]

            # ---- Phase 1a: qpT / kpT projections ----
            for src_r, wi, b_t, dst in (
                (qT_r, 0, bq_t, qp),
                (kT_r, 1, bk_t, kp),
            ):
                for qc in range(QC):
                    xt = io.tile([P, DT, NFQ], bf16, tag="xin")
                    nc.sync.dma_start(
                        out=xt[:], in_=src_r[:, :, qc * NFQ:(qc + 1) * NFQ]
                    )
                    if wi == 0 and qc == 0:
                        # stream the remaining weights behind the first chunk
                        nc.sync.dma_start(out=wall[:, 1], in_=w_rs[1])
                        nc.sync.dma_start(out=wall[:, 2], in_=w_rs[2])
                        nc.sync.dma_start(out=bk_t[:], in_=bk_r)
                        bv_ap = bv[:]
                        nc.sync.dma_start(
                            out=bv_bc[:],
                            in_=bass.AP(
                                tensor=bv_ap.tensor,
                                offset=bv_ap.offset,
                                ap=[[0, P], [1, d]],
                            ),
                        )
                    for et in range(ET):
                        ps = psum.tile([P, NFQ], f32, tag="ps")
                        for dt_ in range(DT):
                            nc.tensor.matmul(
                                ps[:],
                                wall[:, wi, dt_, et * P:(et + 1) * P],
                                xt[:, dt_, :],
                                start=(dt_ == 0),
                                stop=(dt_ == DT - 1),
                            )
                        # bias add (per-partition) + bf16 cast on ScalarE
                        nc.scalar.activation(
                            out=dst[:, et, qc * NFQ:(qc + 1) * NFQ],
                            in_=ps[:],
                            func=mybir.ActivationFunctionType.Identity,
                            bias=b_t[:, et:et + 1],
                        )

            # ---- Phase 1b: vp projection (natural layout) ----
            for sc in range(SC):
                vt = io.tile([P, DT, SCW], bf16, tag="xin")
                nc.sync.dma_start(
                    out=vt[:], in_=vT_r[:, :, sc * SCW:(sc + 1) * SCW]
                )
                for sti in range(SCW // P):
                    st = sc * (SCW // P) + sti
                    for ec in range(EC):
                        ps = psum.tile([P, NFD], f32, tag="ps")
                        for dt_ in range(DT):
                            nc.tensor.matmul(
                                ps[:],
                                vt[:, dt_, sti * P:(sti + 1) * P],
                                wall[:, 2, dt_, ec * NFD:(ec + 1) * NFD],
                                start=(dt_ == 0),
                                stop=(dt_ == DT - 1),
                            )
                        nc.vector.tensor_add(
                            out=vp[:, st, ec * NFD:(ec + 1) * NFD],
                            in0=ps[:],
                            in1=bv_bc[:, ec * NFD:(ec + 1) * NFD],
                        )

            # ---- Phase 2: scoresT -> softmax over q -> probs ----
            # probs reuses the weights' slot (tag W).
            # No max-subtraction: |scores/d| < ~0.3 by construction.
            probs = big.tile([P, ST, s], bf16, tag="W")     # [k, q] per k-tile
            for kt in range(ST):
                partials = small.tile([P, QC], f32, tag="partials")
                for qc in range(QC):
                    ps = psum.tile([P, NFQ], f32, tag="ps")
                    for et in range(ET):
                        nc.tensor.matmul(
                            ps[:],
                            kp[:, et, kt * P:(kt + 1) * P],
                            qp[:, et, qc * NFQ:(qc + 1) * NFQ],
                            start=(et == 0),
                            stop=(et == ET - 1),
                        )
                    nc.scalar.activation(
                        out=probs[:, kt, qc * NFQ:(qc + 1) * NFQ],
                        in_=ps[:],
                        func=mybir.ActivationFunctionType.Exp,
                        scale=1.0 / d,
                        accum_out=partials[:, qc:qc + 1],
                    )
                zsum = small.tile([P, 1], f32, tag="zsum")
                nc.vector.reduce_sum(
                    out=zsum[:], in_=partials[:], axis=mybir.AxisListType.X
                )
                nc.vector.reciprocal(out=rz_all[:, kt:kt + 1], in_=zsum[:])
                # fold 1/Z[k] into vp's k-rows (cheaper than scaling probs)
                nc.vector.tensor_scalar_mul(
                    out=vp[:, kt, :],
                    in0=vp[:, kt, :],
                    scalar1=rz_all[:, kt:kt + 1],
                )

            # ---- Phase 3: attn = probsT.T @ vp ; residual; exp(attn) ----
            # expb reuses qp's slot (tag A).
            expb = big.tile([P, ST, d], bf16, tag="A")      # exp(attn), bf16
            cs_ps = psum1.tile([1, d], f32, tag="cs")       # colsums of exp(attn)
            for st in range(ST):
                qres_t = io.tile([P, d], f32, tag="xin")
                nc.sync.dma_start(out=qres_t[:], in_=qres_r[:, st, :])
                for ec in range(EC):
                    ps = psum.tile([P, NFD], f32, tag="ps")
                    for kt in range(ST):
                        nc.tensor.matmul(
                            ps[:],
                            probs[:, kt, st * P:(st + 1) * P],
                            vp[:, kt, ec * NFD:(ec + 1) * NFD],
                            start=(kt == 0),
                            stop=(kt == ST - 1),
                        )
                    ao = io.tile([P, NFD], f32, tag="ao")
                    nc.vector.tensor_add(
                        out=ao[:],
                        in0=ps[:],
                        in1=qres_t[:, ec * NFD:(ec + 1) * NFD],
                    )
                    nc.sync.dma_start(
                        out=attn_r[:, st, ec * NFD:(ec + 1) * NFD], in_=ao[:]
                    )
                    nc.scalar.activation(
                        out=expb[:, st, ec * NFD:(ec + 1) * NFD],
                        in_=ps[:],
                        func=mybir.ActivationFunctionType.Exp,
                    )
                    nc.tensor.matmul(
                        cs_ps[:, ec * NFD:(ec + 1) * NFD],
                        ones_col[:],
                        expb[:, st, ec * NFD:(ec + 1) * NFD],
                        start=(st == 0),
                        stop=(st == ST - 1),
                    )

            # ---- Phase 3.5: 1/colsum, broadcast across partitions ----
            # approx recip: ~51 ULP, ~5x faster; Z ~ s +- 5% is edge-case-safe
            nc.vector.reciprocal_approx_fast(out=rz2[:], in_=cs_ps[:])
            rzb = psum1.tile([P, d], f32, tag="cs")         # reuses cs_ps bank
            for ec in range(EC):
                nc.tensor.matmul(
                    rzb[:, ec * NFD:(ec + 1) * NFD],
                    ones_row[:],
                    rz2[:, ec * NFD:(ec + 1) * NFD],
                    start=True,
                    stop=True,
                )

            # ---- Phase 4: attn_w = exp(attn) * (1/colsum) ----
            # 4-deep staging carved from kp's dead slot so the multiply/DMA
            # chain pipelines (2 io-pool slots paced the writes at ~2.4us/tile)
            NAW = min(4, ST)
            aw_all = big.tile([P, NAW, d], f32, tag="B")
            for st in range(ST):
                aw = aw_all[:, st % NAW, :]
                nc.vector.tensor_mul(out=aw, in0=expb[:, st, :], in1=rzb[:])
                nc.sync.dma_start(out=attnw_r[:, st, :], in_=aw)

    return nc


def _host_prep(q, k, v, Wq, bq, Wk, bk, Wv, bv):
    """Shard over batch and pre-transpose/cast on host."""
    bf16 = ml_dtypes.bfloat16
    q = np.asarray(q, dtype=np.float32)
    k = np.asarray(k, dtype=np.float32)
    v = np.asarray(v, dtype=np.float32)
    wqT = np.asarray(Wq, dtype=np.float32).T.astype(bf16)  # [d, e]
    wkT = np.asarray(Wk, dtype=np.float32).T.astype(bf16)
    wvT = np.asarray(Wv, dtype=np.float32).T.astype(bf16)
    bq = np.ascontiguousarray(np.asarray(bq, dtype=np.float32))
    bk = np.ascontiguousarray(np.asarray(bk, dtype=np.float32))
    bv = np.ascontiguousarray(np.asarray(bv, dtype=np.float32))

    in_maps = []
    for i in range(B):
        in_maps.append(
            {
                "qT": q[i].T.astype(bf16),
                "kT": k[i].T.astype(bf16),
                "vT": v[i].T.astype(bf16),
                "wqT": wqT,
                "wkT": wkT,
                "wvT": wvT,
                "bq": bq,
                "bk": bk,
                "bv": bv,
                "qres": np.ascontiguousarray(q[i]),
            }
        )
    return in_maps


_CACHED_NC = None


def kernel(q, k, v, Wq, bq, Wk, bk, Wv, bv):
    global _CACHED_NC
    from concourse import bass_utils

    in_maps = _host_prep(q, k, v, Wq, bq, Wk, bk, Wv, bv)
    if _CACHED_NC is None:
        _CACHED_NC = build_nc()
        _CACHED_NC.finalize()  # bacc passes (reg alloc, wait splitting)
    res = bass_utils.run_bass_kernel_spmd(
        _CACHED_NC, in_maps, core_ids=list(range(B))
    )
    attn = np.stack([np.asarray(res.results[i]["attn"]) for i in range(B)])
    attn_w = np.stack([np.asarray(res.results[i]["attn_w"]) for i in range(B)])
    return attn.astype(np.float32), attn_w.astype(np.float32)



# revision 7
# speedup vs baseline: 1.7990x; 1.7990x over previous
"""Trainium2 Bass kernel for nn_MultiHeadAttention_66872640799208.

Math (per batch element b, S=2048, D=1024):
    qp = q @ Wq.T + bq ; kp = k @ Wk.T + bk ; vp = v @ Wv.T + bv
    scores = qp @ kp.T / D
    probs  = softmax(scores, axis=q)          # over the QUERY axis
    attn   = probs @ vp
    attn_w = softmax(attn, axis=q)            # over the sequence axis
    out    = (attn + q, attn_w)

Sharding: data-parallel over batch B=8 -> one batch element per NeuronCore,
no collectives.

All five big matmuls run in fp8e4m3 with MatmulPerfMode.DoubleRow: each
instruction contracts K=256 (two 128-partition tiles packed along the free
dim of both operands) at 0.5 cycles per output row -- 4x bf16 MACs/cycle in
the cost model. Layouts (host pre-transposes, casts to fp8):
  - qT/kT/vT [D, S] fp8 feed the projections (contraction over d),
  - qpT/kpT are produced in [e, s] fp8 so scoresT = kpT.T @ qpT has the
    softmax axis (q) on the free dimension,
  - vp is produced in natural [s, e] bf16, then quantized to fp8 with the
    softmax-1 normalization folded in as (2048/Z_k) ~= 1.0 (fp8-safe; the
    leftover global 1/2048 moves into the exp scale of softmax #2 and a
    host-side divide of the residual output -- both exact).
The attn matmul psum therefore carries 2048*attn; the residual add uses
host-prescaled 2048*q (bf16) and the host divides the output by 2048.
The second softmax (over q, the partition axis) uses a ones-vector PE
matmul for column sums (bf16 expb -- fp8 there would put its quantization
noise directly on attn_w) and a K=1 PE matmul to broadcast 1/Z.

SBUF per partition: wall 24K / probs 32K (tag W), qp 16K / expb 32K
(tag A), kp 16K / aw 8K (tag B), vp16 32K, vp8 16K, io ~20K: ~150KB.
"""

import sys

if "/opt/trn_rl_repo" not in sys.path:
    sys.path.insert(0, "/opt/trn_rl_repo")

import numpy as np
import ml_dtypes

B = 8
S = 2048
D = 1024
P = 128


def build_nc(s=S, d=D):
    """Build the single-core Bass program (SPMD: identical on all cores)."""
    import concourse.bass as bass
    import concourse.tile as tile
    from concourse import bacc, mybir

    bf16 = mybir.dt.bfloat16
    fp8 = mybir.dt.float8e4
    f32 = mybir.dt.float32
    DR = mybir.MatmulPerfMode.DoubleRow

    DT = d // P          # contraction tiles for projections
    DH = DT // 2         # DoubleRow pairs over d
    ET = d // P          # e (feature) tiles
    EH = ET // 2         # DoubleRow pairs over e
    ST = s // P          # sequence tiles
    SH = ST // 2         # DoubleRow pairs over s (attn contraction)
    NFQ = min(512, s)    # matmul moving free-dim over q
    QC = s // NFQ        # q chunks
    NFD = min(512, d)    # matmul moving free-dim over d/e
    EC = d // NFD        # e chunks
    SCW = min(512, s)    # vT stream chunk width (in s)
    SC = s // SCW

    nc = bacc.Bacc("TRN2")

    qT = nc.dram_tensor("qT", [d, s], fp8, kind="ExternalInput")
    kT = nc.dram_tensor("kT", [d, s], fp8, kind="ExternalInput")
    vT = nc.dram_tensor("vT", [d, s], fp8, kind="ExternalInput")
    wqT = nc.dram_tensor("wqT", [d, d], fp8, kind="ExternalInput")  # [d, e]
    wkT = nc.dram_tensor("wkT", [d, d], fp8, kind="ExternalInput")
    wvT = nc.dram_tensor("wvT", [d, d], fp8, kind="ExternalInput")
    bq = nc.dram_tensor("bq", [d], f32, kind="ExternalInput")
    bk = nc.dram_tensor("bk", [d], f32, kind="ExternalInput")
    bv = nc.dram_tensor("bv", [d], f32, kind="ExternalInput")
    qres = nc.dram_tensor("qres", [s, d], bf16, kind="ExternalInput")  # 2048*q
    attn_o = nc.dram_tensor("attn", [s, d], bf16, kind="ExternalOutput")
    attnw_o = nc.dram_tensor("attn_w", [s, d], bf16, kind="ExternalOutput")

    qT_r = qT[:].rearrange("(dt p) s -> p dt s", p=P)
    kT_r = kT[:].rearrange("(dt p) s -> p dt s", p=P)
    vT_r = vT[:].rearrange("(dt p) s -> p dt s", p=P)
    w_rs = [
        w[:].rearrange("(dt p) e -> p dt e", p=P) for w in (wqT, wkT, wvT)
    ]
    bq_r = bq[:].rearrange("(t p) -> p t", p=P)
    bk_r = bk[:].rearrange("(t p) -> p t", p=P)
    qres_r = qres[:].rearrange("(st p) d -> p st d", p=P)
    attn_r = attn_o[:].rearrange("(st p) d -> p st d", p=P)
    attnw_r = attnw_o[:].rearrange("(st p) d -> p st d", p=P)

    with tile.TileContext(nc) as tc:
        with (
            tc.tile_pool(name="consts", bufs=1) as consts,
            tc.tile_pool(name="big", bufs=1) as big,
            tc.tile_pool(name="io", bufs=2) as io,
            tc.tile_pool(name="small", bufs=4) as small,
            tc.tile_pool(name="psum", bufs=4, space="PSUM") as psum,
            tc.tile_pool(name="psum1", bufs=1, space="PSUM") as psum1,
        ):
            # ---- constants. Weights/biases ride the scalar HWDGE queue so
            # the sync queue is free for the qT/kT/vT stream: the PE's first
            # matmul only waits on wall0 + the first q-chunk, in parallel. ----
            wall = big.tile([P, 3, DT, d], fp8, tag="W")   # wq|wk|wv
            nc.scalar.dma_start(out=wall[:, 0], in_=w_rs[0])
            bq_t = consts.tile([P, ET], f32)
            nc.scalar.dma_start(out=bq_t[:], in_=bq_r)
            bk_t = consts.tile([P, ET], f32)
            bv_bc = consts.tile([P, d], f32)
            ones_col = consts.tile([P, 1], bf16)   # lhsT for column sums (K=P, M=1)
            nc.vector.memset(ones_col[:], 1.0)
            rz_all = consts.tile([P, ST], f32)     # per-k-row 1/Z of softmax #1
            rz2 = consts.tile([1, d], f32)         # 1/colsum of softmax #2
            rzb_sb = consts.tile([P, d], bf16)     # bcast 1/colsum, bf16
            ones_row = consts.tile([1, P], f32)    # lhsT for 1/Z broadcast (K=1)
            nc.vector.memset(ones_row[:], 1.0)

            qp = big.tile([P, ET, s], fp8, tag="A")         # qpT: [e, q]
            kp = big.tile([P, ET, s], fp8, tag="B")         # kpT: [e, k]
            vp16 = big.tile([P, ST, d], bf16, tag="V2")     # natural [s, e]
            vp8 = big.tile([P, ST, d], fp8, tag="V")        # *(2048/Z_k)

            # ---- Phase 1a: qpT / kpT projections ----
            for src_r, wi, b_t, dst in (
                (qT_r, 0, bq_t, qp),
                (kT_r, 1, bk_t, kp),
            ):
                for qc in range(QC):
                    xt = io.tile([P, DT, NFQ], fp8, tag="xin")
                    nc.sync.dma_start(
                        out=xt[:], in_=src_r[:, :, qc * NFQ:(qc + 1) * NFQ]
                    )
                    if wi == 0 and qc == 0:
                        # remaining weights stream on their own (scalar) queue
                        nc.scalar.dma_start(out=wall[:, 1], in_=w_rs[1])
                        nc.scalar.dma_start(out=wall[:, 2], in_=w_rs[2])
                        nc.scalar.dma_start(out=bk_t[:], in_=bk_r)
                        bv_ap = bv[:]
                        nc.scalar.dma_start(
                            out=bv_bc[:],
                            in_=bass.AP(
                                tensor=bv_ap.tensor,
                                offset=bv_ap.offset,
                                ap=[[0, P], [1, d]],
                            ),
                        )
                    for et in range(ET):
                        ps = psum.tile([P, NFQ], f32, tag="ps")
                        for j in range(DH):
                            nc.tensor.matmul(
                                ps[:],
                                wall[:, wi, 2 * j:2 * j + 2, et * P:(et + 1) * P],
                                xt[:, 2 * j:2 * j + 2, :],
                                start=(j == 0),
                                stop=(j == DH - 1),
                                perf_mode=DR,
                            )
                        # bias add (per-partition) + fp8 cast on ScalarE
                        nc.scalar.activation(
                            out=dst[:, et, qc * NFQ:(qc + 1) * NFQ],
                            in_=ps[:],
                            func=mybir.ActivationFunctionType.Identity,
                            bias=b_t[:, et:et + 1],
                        )

            # ---- Phase 1b: vp projection (natural layout, bf16) ----
            for sc in range(SC):
                vt = io.tile([P, DT, SCW], fp8, tag="xin")
                nc.sync.dma_start(
                    out=vt[:], in_=vT_r[:, :, sc * SCW:(sc + 1) * SCW]
                )
                for sti in range(SCW // P):
                    st = sc * (SCW // P) + sti
                    for ec in range(EC):
                        ps = psum.tile([P, NFD], f32, tag="ps")
                        for j in range(DH):
                            nc.tensor.matmul(
                                ps[:],
                                vt[:, 2 * j:2 * j + 2, sti * P:(sti + 1) * P],
                                wall[:, 2, 2 * j:2 * j + 2,
                                     ec * NFD:(ec + 1) * NFD],
                                start=(j == 0),
                                stop=(j == DH - 1),
                                perf_mode=DR,
                            )
                        nc.vector.tensor_add(
                            out=vp16[:, st, ec * NFD:(ec + 1) * NFD],
                            in0=ps[:],
                            in1=bv_bc[:, ec * NFD:(ec + 1) * NFD],
                        )

            # ---- Phase 2: scoresT -> softmax over q -> probs (fp8) ----
            # probs reuses the weights' slot (tag W).
            # No max-subtraction: |scores/d| < ~0.3 by construction.
            probs = big.tile([P, ST, s], fp8, tag="W")      # [k, q] per k-tile
            for kt in range(ST):
                for qc in range(QC):
                    ps = psum.tile([P, NFQ], f32, tag="ps")
                    for j in range(EH):
                        nc.tensor.matmul(
                            ps[:],
                            kp[:, 2 * j:2 * j + 2, kt * P:(kt + 1) * P],
                            qp[:, 2 * j:2 * j + 2, qc * NFQ:(qc + 1) * NFQ],
                            start=(j == 0),
                            stop=(j == EH - 1),
                            perf_mode=DR,
                        )
                    nc.scalar.activation(
                        out=probs[:, kt, qc * NFQ:(qc + 1) * NFQ],
                        in_=ps[:],
                        func=mybir.ActivationFunctionType.Exp,
                        scale=1.0 / d,
                    )
                # Z from the quantized probs on the (underused) DVE — keeps
                # the scalar engine off the ACTIVATION_READ_ACCUMULATOR path
                zsum = small.tile([P, 1], f32, tag="zsum")
                nc.vector.reduce_sum(
                    out=zsum[:], in_=probs[:, kt, :], axis=mybir.AxisListType.X
                )
                nc.vector.reciprocal(out=rz_all[:, kt:kt + 1], in_=zsum[:])
                # quantize vp to fp8 with (2048/Z_k) folded into its k-rows:
                # ~1.0 so the fp8 range is preserved; the global 1/2048 is
                # repaid at the attn psum (exp scale + host divide).
                nc.vector.tensor_scalar(
                    out=vp8[:, kt, :],
                    in0=vp16[:, kt, :],
                    scalar1=rz_all[:, kt:kt + 1],
                    scalar2=float(s),
                    op0=mybir.AluOpType.mult,
                    op1=mybir.AluOpType.mult,
                )

            # ---- Phase 3: 2048*attn = probsT.T @ vp8 ; residual; exp ----
            # expb reuses qp's slot (tag A).
            expb = big.tile([P, ST, d], bf16, tag="A")      # exp(attn), bf16
            cs_ps = psum1.tile([1, d], f32, tag="cs")       # colsums of exp(attn)
            for st in range(ST):
                qres_t = io.tile([P, d], bf16, tag="xin")
                # gpsimd SWDGE queue: keeps the sync queue free for attn out
                nc.gpsimd.dma_start(out=qres_t[:], in_=qres_r[:, st, :])
                ao = io.tile([P, d], bf16, tag="ao")
                for ec in range(EC):
                    ps = psum.tile([P, NFD], f32, tag="ps")
                    for j in range(SH):
                        nc.tensor.matmul(
                            ps[:],
                            probs[:, 2 * j:2 * j + 2, st * P:(st + 1) * P],
                            vp8[:, 2 * j:2 * j + 2, ec * NFD:(ec + 1) * NFD],
                            start=(j == 0),
                            stop=(j == SH - 1),
                            perf_mode=DR,
                        )
                    nc.vector.tensor_add(
                        out=ao[:, ec * NFD:(ec + 1) * NFD],
                        in0=ps[:],
                        in1=qres_t[:, ec * NFD:(ec + 1) * NFD],
                    )
                    nc.scalar.activation(
                        out=expb[:, st, ec * NFD:(ec + 1) * NFD],
                        in_=ps[:],
                        func=mybir.ActivationFunctionType.Exp,
                        scale=1.0 / s,
                    )
                    nc.tensor.matmul(
                        cs_ps[:, ec * NFD:(ec + 1) * NFD],
                        ones_col[:],
                        expb[:, st, ec * NFD:(ec + 1) * NFD],
                        start=(st == 0),
                        stop=(st == ST - 1),
                    )
                nc.sync.dma_start(out=attn_r[:, st, :], in_=ao[:])

            # ---- Phase 3.5: 1/colsum, broadcast across partitions ----
            # approx recip: ~51 ULP, ~5x faster; Z ~ s +- 5% is edge-case-safe
            nc.vector.reciprocal_approx_fast(out=rz2[:], in_=cs_ps[:])
            rzb = psum1.tile([P, d], f32, tag="cs")         # reuses cs_ps bank
            for ec in range(EC):
                nc.tensor.matmul(
                    rzb[:, ec * NFD:(ec + 1) * NFD],
                    ones_row[:],
                    rz2[:, ec * NFD:(ec + 1) * NFD],
                    start=True,
                    stop=True,
                )
            # bf16 copy so the phase-4 multiplies run at 2x 16-bit DVE rate
            nc.scalar.copy(out=rzb_sb[:], in_=rzb[:])

            # ---- Phase 4: attn_w = exp(attn) * (1/colsum) ----
            # Batched: one [P, NG*d] multiply + one DMA per group of NG
            # s-tiles, double-buffered, DMAs alternating sync/scalar HWDGE
            # queues so the two 1MB writes overlap. rzb is read through a
            # stride-0 AP to broadcast it across the NG tiles of a group.
            NG = min(4, ST)
            aw_all = big.tile([P, 2, NG, d], bf16, tag="B")
            rz_ap = rzb_sb[:]
            rz_bc = bass.AP(
                tensor=rz_ap.tensor,
                offset=rz_ap.offset,
                ap=[rz_ap.ap[0], [0, NG], [1, d]],
            )
            for g in range(ST // NG):
                aw = aw_all[:, g % 2]
                nc.vector.tensor_mul(
                    out=aw, in0=expb[:, g * NG:(g + 1) * NG, :], in1=rz_bc
                )
                eng = nc.sync if g % 2 == 0 else nc.scalar
                eng.dma_start(out=attnw_r[:, g * NG:(g + 1) * NG, :], in_=aw)

    return nc


def _host_prep(q, k, v, Wq, bq, Wk, bk, Wv, bv):
    """Shard over batch and pre-transpose/cast on host."""
    fp8 = ml_dtypes.float8_e4m3
    bf16 = ml_dtypes.bfloat16
    q = np.asarray(q, dtype=np.float32)
    k = np.asarray(k, dtype=np.float32)
    v = np.asarray(v, dtype=np.float32)
    wqT = np.asarray(Wq, dtype=np.float32).T.astype(fp8)  # [d, e]
    wkT = np.asarray(Wk, dtype=np.float32).T.astype(fp8)
    wvT = np.asarray(Wv, dtype=np.float32).T.astype(fp8)
    bq = np.ascontiguousarray(np.asarray(bq, dtype=np.float32))
    bk = np.ascontiguousarray(np.asarray(bk, dtype=np.float32))
    bv = np.ascontiguousarray(np.asarray(bv, dtype=np.float32))

    in_maps = []
    for i in range(B):
        in_maps.append(
            {
                "qT": q[i].T.astype(fp8),
                "kT": k[i].T.astype(fp8),
                "vT": v[i].T.astype(fp8),
                "wqT": wqT,
                "wkT": wkT,
                "wvT": wvT,
                "bq": bq,
                "bk": bk,
                "bv": bv,
                "qres": (q[i] * float(S)).astype(bf16),
            }
        )
    return in_maps


def _host_post(attn_raw, attnw_raw):
    """Undo the 2048x psum scaling and widen to f32."""
    attn = attn_raw.astype(np.float32) * (1.0 / float(S))
    attn_w = attnw_raw.astype(np.float32)
    return attn, attn_w


_CACHED_NC = None


def kernel(q, k, v, Wq, bq, Wk, bk, Wv, bv):
    global _CACHED_NC
    from concourse import bass_utils

    in_maps = _host_prep(q, k, v, Wq, bq, Wk, bk, Wv, bv)
    if _CACHED_NC is None:
        _CACHED_NC = build_nc()
        _CACHED_NC.finalize()  # bacc passes (reg alloc, wait splitting)
    res = bass_utils.run_bass_kernel_spmd(
        _CACHED_NC, in_maps, core_ids=list(range(B))
    )
    attn = np.stack([np.asarray(res.results[i]["attn"]) for i in range(B)])
    attn_w = np.stack([np.asarray(res.results[i]["attn_w"]) for i in range(B)])
    return _host_post(attn, attn_w)
